# revision 19
# baseline (speedup 1.0000x reference)
"""MinLSTM fused kernel for TRN2 (8 NeuronCores, batch-parallel), bf16.

Math (equivalent to the reference's log-space form):
    zf = x@Wf+bf ; zi = x@Wi+bi ; zh = x@Wh+bh
    Ef = exp(-zf)               # 1/sigmoid(zf) = 1 + Ef
    si = sigmoid(zi) ; sh = sigmoid(zh)
    g  = max(zh + bh + 0.5, sh)
    p  = (1 + Ef) * si          # = si/sf
    S  = 0.5 + cumsum(p*g, axis=time)
    out[:, 0, :]  = 0.5
    out[:, t+1, :] = S[t] / (1 + p[t])

Per core (one batch element): bf16 GEMMs z^T [512h, 4096t] (stationary =
W chunk, moving = x^T chunk, 4 rotating PSUM slots of [128,1024]).
Elementwise in [H-partition, T-free]: ACT does Exp/Sigmoid/Reciprocal in
table-batched phases per h-chunk pair (exp -> sig -> recip); DVE does the
g/p STTs and the fp32-accum scan (bf16 out); Pool does the u and o
tensor muls (bf16). Output written bf16, host transposes + upconverts.
"""
import numpy as np

_CACHE = {}

B, T, D, H = 8, 4096, 512, 512
NCORES = 8
N_HC = H // 128       # 4 h-chunks
N_D = D // 128        # 4 contraction chunks
N_TC = 4              # 1024-wide T chunks per h for GEMM+ACT
TCW = 1024
HALF = 2048           # p/u/scan/r2/o granularity
XW_COLS = 3 * H + T   # 5632


def _install_tilefix():
    """This walrus build accepts only ONE sync wait per hardware instruction;
    Tile can emit several. Spill extras onto injected single-wait drains."""
    import concourse.tile as tile
    from concourse import mybir
    from concourse.vector_clock import ScopedClock

    if getattr(tile.TileContext, "_minlstm_patched", False):
        return
    orig_lower = tile.TileContext._lower_ordered_insts

    def _spill_waits(self, ordered):
        nc = self.nc
        for bb_name, insts in ordered.items():
            out = []
            for inst in insts:
                si = inst.sync_info
                if si is not None and len(si.on_wait) > 1 and inst.engine is not None:
                    waits = list(si.on_wait)
                    for w in waits[:-1]:
                        d = mybir.InstDrain(
                            name=nc.get_next_instruction_name(),
                            ins=[], outs=[], bass_is_fusable=False,
                            sync_info=mybir.SyncInfo(on_wait=[w], on_update=[]),
                        )
                        d.engine = inst.engine
                        out.append(d)
                    si.on_wait = [waits[-1]]
                out.append(inst)
            insts[:] = out
        return ordered

    def _patched_lower(self, ordered):
        return orig_lower(self, _spill_waits(self, ordered))

    def _split_drain_and_barrier(self, tick_clock, wait_clock):
        drain_inst = self.nc.sync.drain()
        wait_clock.add_sem_waits(
            drain_inst.ins, ScopedClock({None: tick_clock.global_clock})
        )
        si = drain_inst.ins.sync_info
        if si is not None and len(si.on_wait) > 1:
            waits = list(si.on_wait)
            si.on_wait = [waits[0]]
            for w in waits[1:]:
                extra = self.nc.sync.drain()
                esi = extra.ins.sync_info
                if esi is None:
                    extra.ins.sync_info = mybir.SyncInfo(on_wait=[w], on_update=[])
                else:
                    esi.on_wait = [w]
        self.nc.all_engine_barrier()
        assert self.sems is not None
        popped = self.nc._tile_sem_poison_stack.pop()
        assert popped is self._sem_poison
        self.nc.clear_and_free_semaphores(list(self.sems.allocated().values()))
        self.nc.all_engine_barrier()

    tile.TileContext._lower_ordered_insts = _patched_lower
    tile.TileContext._drain_and_barrier = _split_drain_and_barrier
    tile.TileContext._minlstm_patched = True


def _build():
    import concourse.bass as bass
    import concourse.tile as tile
    from concourse import mybir
    from concourse.tile_rust import add_dep_helper

    _install_tilefix()

    f32 = mybir.dt.float32
    bf16 = mybir.dt.bfloat16
    AF = mybir.ActivationFunctionType
    ALU = mybir.AluOpType

    nc = bass.Bass("TRN2", target_bir_lowering=False, debug=False,
                   num_devices=NCORES)

    xw_d = nc.dram_tensor("xw", [D, XW_COLS], bf16, kind="ExternalInput").ap()
    bias_d = nc.dram_tensor("biases", [128, 16], f32, kind="ExternalInput").ap()
    out_d = nc.dram_tensor("out", [H, T], bf16, kind="ExternalOutput").ap()

    prev_act = [None]

    def act_raw(out, in_, func, bias=0.0, scale=1.0):
        eng = nc.scalar
        inputs = [eng.lower_ap(in_)]
        for arg in (bias, scale, 0.0):
            if isinstance(arg, bass.AP):
                inputs.append(eng.lower_ap(arg))
            else:
                inputs.append(
                    mybir.ImmediateValue(dtype=f32, value=float(arg))
                )
        i = eng.add_instruction(
            mybir.InstActivation(
                name=nc.get_next_instruction_name(),
                func=func, ins=inputs, outs=[eng.lower_ap(out)],
            )
        )
        if prev_act[0] is not None:
            add_dep_helper(i.ins, prev_act[0].ins, sync=False,
                           reason="ACT table-set order")
        prev_act[0] = i
        return i

    with tile.TileContext(nc) as tc:
        with (
            tc.tile_pool(name="xwp", bufs=1) as xwp,
            tc.tile_pool(name="cons", bufs=1) as cons,
            tc.tile_pool(name="ps", bufs=4, space="PSUM") as ps,
            tc.tile_pool(name="grid", bufs=2) as grid,
            tc.tile_pool(name="shp", bufs=4) as shp,
            tc.tile_pool(name="pp", bufs=4) as pp,
            tc.tile_pool(name="up", bufs=4) as up,
            tc.tile_pool(name="Sp", bufs=4) as Sp,
            tc.tile_pool(name="S1kp", bufs=8) as S1kp,
            tc.tile_pool(name="rp", bufs=4) as rp,
            tc.tile_pool(name="op", bufs=4) as op,
        ):
            xw = [
                xwp.tile([128, XW_COLS], bf16, tag=f"xw{d}", name=f"xw{d}")
                for d in range(N_D)
            ]
            # Wf first, then x^T tc0, then Wi/Wh, then remaining x^T chunks
            for d in range(N_D):
                nc.sync.dma_start(
                    xw[d][:, 0:H], xw_d[128 * d:128 * (d + 1), 0:H])
            for d in range(N_D):
                c0 = 3 * H
                nc.sync.dma_start(
                    xw[d][:, c0:c0 + TCW],
                    xw_d[128 * d:128 * (d + 1), c0:c0 + TCW])
            for d in range(N_D):
                nc.sync.dma_start(
                    xw[d][:, H:3 * H], xw_d[128 * d:128 * (d + 1), H:3 * H])
            for tcol in range(1, N_TC):
                c0 = 3 * H + TCW * tcol
                for d in range(N_D):
                    nc.sync.dma_start(
                        xw[d][:, c0:c0 + TCW],
                        xw_d[128 * d:128 * (d + 1), c0:c0 + TCW])
            bt = cons.tile([128, 16], f32, tag="bt")
            nc.sync.dma_start(bt[:], bias_d[:])
            zero1 = cons.tile([128, 8], f32, tag="zero1")
            nc.vector.memset(zero1[:], 0.0)
            zb = zero1[:, 0:1].broadcast_to([128, HALF])
            zb1k = zero1[:, 0:1].broadcast_to([128, TCW])

            def gemm(gate, h, tcol, name):
                z = ps.tile([128, TCW], f32, tag="z", name=name)
                for half in range(2):
                    sl = slice(512 * half, 512 * (half + 1))
                    t0 = 3 * H + TCW * tcol + 512 * half
                    for d in range(N_D):
                        nc.tensor.matmul(
                            z[:, sl], xw[d][:, 512 * gate + 128 * h:
                                            512 * gate + 128 * h + 128],
                            xw[d][:, t0:t0 + 512],
                            start=(d == 0), stop=(d == N_D - 1),
                        )
                return z

            backlog = []

            def drain(n):
                for _ in range(min(n, len(backlog))):
                    backlog.pop(0)()

            for pair in range(2):
                hs = (2 * pair, 2 * pair + 1)
                last = pair == 1

                Ef = {}
                si = {}
                g = {}
                for h in hs:
                    Ef[h] = grid.tile([128, T], bf16, tag="Ef", name=f"Ef{h}")
                    si[h] = grid.tile([128, T], bf16, tag="si", name=f"si{h}")
                    g[h] = grid.tile([128, T], bf16, tag="g", name=f"g{h}")

                # ---- EXP phase: zf GEMMs -> Ef = exp(-zf) ----
                for h in hs:
                    nbf_ap = bt[:, h:h + 1]            # -bf
                    for tcol in range(N_TC):
                        z = gemm(0, h, tcol, f"zf{h}_{tcol}")
                        act_raw(Ef[h][:, TCW * tcol:TCW * (tcol + 1)], z[:],
                                AF.Exp, bias=nbf_ap, scale=-1.0)
                        if tcol % 2 == 1:
                            drain(1)    # prev-pair scan/out piece

                # ---- SIG phase + per-pair postlude ----
                pt = {h: [] for h in hs}
                ut = {h: [] for h in hs}
                St = {h: [] for h in hs}
                rt = {h: [] for h in hs}

                def sig_section(h):
                    bi_ap = bt[:, 4 + h:5 + h]
                    bg_ap = bt[:, 8 + h:9 + h]         # bh + 0.5
                    bh_ap = bt[:, 12 + h:13 + h]
                    for tcol in range(N_TC):
                        zi = gemm(1, h, tcol, f"zi{h}_{tcol}")
                        act_raw(si[h][:, TCW * tcol:TCW * (tcol + 1)], zi[:],
                                AF.Sigmoid, bias=bi_ap)
                        zh = gemm(2, h, tcol, f"zh{h}_{tcol}")
                        sh = shp.tile([128, TCW], bf16, tag="sh",
                                      name=f"sh{h}_{tcol}")
                        act_raw(sh[:], zh[:], AF.Sigmoid, bias=bh_ap)
                        nc.vector.scalar_tensor_tensor(
                            out=g[h][:, TCW * tcol:TCW * (tcol + 1)],
                            in0=zh[:], scalar=bg_ap, in1=sh[:],
                            op0=ALU.add, op1=ALU.max,
                        )
                        if tcol % 2 == 1:
                            drain(1)

                def p_and_u(h):
                    for half in range(2):
                        sl = slice(HALF * half, HALF * (half + 1))
                        p = pp.tile([128, HALF], bf16, tag="p",
                                    name=f"p{h}_{half}")
                        nc.vector.scalar_tensor_tensor(
                            out=p[:], in0=Ef[h][:, sl], scalar=1.0,
                            in1=si[h][:, sl], op0=ALU.add, op1=ALU.mult,
                        )
                        pt[h].append(p)
                    for k in range(N_TC):
                        sl = slice(TCW * k, TCW * (k + 1))
                        u = up.tile([128, TCW], bf16, tag="u1k",
                                    name=f"u1k{h}_{k}")
                        nc.vector.tensor_tensor(
                            out=u[:],
                            in0=pt[h][k // 2][:, TCW * (k % 2):TCW * (k % 2 + 1)],
                            in1=g[h][:, sl], op=ALU.mult)
                        ut[h].append(u)

                def scan_piece(h, k):
                    S = S1kp.tile([128, TCW], bf16, tag="S1k",
                                  name=f"S{h}_{k}")
                    init = 0.5 if k == 0 else St[h][-1][:, TCW - 1:TCW]
                    nc.vector.tensor_tensor_scan(
                        S[:], zb1k, ut[h][k][:], init, ALU.add, ALU.add)
                    St[h].append(S)

                def r2_section(h):
                    for half in range(2):
                        r2 = rp.tile([128, HALF], bf16, tag="r2",
                                     name=f"r2_{h}_{half}")
                        act_raw(r2[:], pt[h][half][:], AF.Reciprocal,
                                bias=1.0)
                        rt[h].append(r2)

                def out_piece(h, k, eng):
                    o = op.tile([128, TCW], bf16, tag="o", name=f"o{h}_{k}")
                    r2sl = rt[h][k // 2][:, TCW * (k % 2):TCW * (k % 2 + 1)]
                    if eng == "v":
                        nc.vector.tensor_tensor(out=o[:], in0=St[h][k][:],
                                                in1=r2sl, op=ALU.mult)
                    else:
                        nc.gpsimd.tensor_tensor(out=o[:], in0=St[h][k][:],
                                                in1=r2sl, op=ALU.mult)
                    nc.sync.dma_start(
                        out_d[128 * h:128 * (h + 1), TCW * k:TCW * (k + 1)],
                        o[:],
                    )

                if not last:
                    for h in hs:
                        sig_section(h)
                    for h in hs:
                        p_and_u(h)
                    for h in hs:
                        r2_section(h)

                    def mk_piece(h, k, ut=ut, St=St, rt=rt):
                        def emit():
                            S = S1kp.tile([128, TCW], bf16, tag="S1k",
                                          name=f"S{h}_{k}")
                            init = (0.5 if k == 0
                                    else St[h][-1][:, TCW - 1:TCW])
                            nc.vector.tensor_tensor_scan(
                                S[:], zb1k, ut[h][k][:], init,
                                ALU.add, ALU.add)
                            St[h].append(S)
                            o = op.tile([128, TCW], bf16, tag="o",
                                        name=f"o{h}_{k}")
                            r2sl = rt[h][k // 2][:, TCW * (k % 2):
                                                 TCW * (k % 2 + 1)]
                            nc.gpsimd.tensor_tensor(out=o[:], in0=St[h][k][:],
                                                    in1=r2sl, op=ALU.mult)
                            nc.sync.dma_start(
                                out_d[128 * h:128 * (h + 1),
                                      TCW * k:TCW * (k + 1)],
                                o[:],
                            )
                        return emit

                    for h in hs:
                        for k in range(N_TC):
                            backlog.append(mk_piece(h, k))
                else:
                    h2, h3 = hs
                    # h2: full chain inline; its scans run under h3's GEMMs,
                    # its outs on Pool in parallel
                    sig_section(h2)
                    p_and_u(h2)
                    for k in range(N_TC):
                        scan_piece(h2, k)
                    r2_section(h2)        # covered by h3's GEMMs
                    for k in range(N_TC):
                        out_piece(h2, k, "g")
                    # h3: pieces chase the final GEMMs; only its short chain
                    # plus r2 remains after the last matmul
                    sig_section(h3)
                    p_and_u(h3)
                    for k in range(N_TC):
                        scan_piece(h3, k)
                    r2_section(h3)
                    for k in range(N_TC):
                        out_piece(h3, k, "v")
            drain(len(backlog))
    return nc


def _get_nc():
    if "nc" not in _CACHE:
        _CACHE["nc"] = _build()
    return _CACHE["nc"]


def _make_in_maps(x, Wf, bf, Wi, bi, Wh, bh):
    import ml_dtypes
    bft = ml_dtypes.bfloat16

    x = np.asarray(x, dtype=np.float32)
    W_all = np.concatenate(
        [np.asarray(Wf), np.asarray(Wi), np.asarray(Wh)], axis=1
    ).astype(bft)

    bf32 = np.asarray(bf, dtype=np.float32)
    bi32 = np.asarray(bi, dtype=np.float32)
    bh32 = np.asarray(bh, dtype=np.float32)
    biases = np.zeros((128, 16), dtype=np.float32)
    biases[:, 0:4] = (-bf32).reshape(N_HC, 128).T
    biases[:, 4:8] = bi32.reshape(N_HC, 128).T
    biases[:, 8:12] = (bh32 + np.float32(0.5)).reshape(N_HC, 128).T
    biases[:, 12:16] = bh32.reshape(N_HC, 128).T

    in_maps = []
    for c in range(NCORES):
        xT = np.ascontiguousarray(x[c].T).astype(bft)
        xw = np.concatenate([W_all, xT], axis=1)
        in_maps.append({"xw": xw, "biases": biases})
    return in_maps


def kernel(x, Wf, bf, Wi, bi, Wh, bh):
    from concourse.bass_utils import run_bass_kernel_spmd

    in_maps = _make_in_maps(x, Wf, bf, Wi, bi, Wh, bh)
    nc = _get_nc()
    res = run_bass_kernel_spmd(nc, in_maps, list(range(NCORES)))

    out = np.empty((B, T + 1, H), dtype=np.float32)
    out[:, 0, :] = np.float32(0.5)
    for c in range(NCORES):
        out[c, 1:, :] = np.asarray(res.results[c]["out"]).astype(np.float32).T
    return out


# revision 20
# speedup vs baseline: 1.0133x; 1.0133x over previous
"""MinLSTM fused kernel for TRN2 (8 NeuronCores, batch-parallel), bf16.

Math (equivalent to the reference's log-space form):
    zf = x@Wf+bf ; zi = x@Wi+bi ; zh = x@Wh+bh
    Ef = exp(-zf)               # 1/sigmoid(zf) = 1 + Ef
    si = sigmoid(zi) ; sh = sigmoid(zh)
    g  = max(zh + bh + 0.5, sh)
    p  = (1 + Ef) * si          # = si/sf
    S  = 0.5 + cumsum(p*g, axis=time)
    out[:, 0, :]  = 0.5
    out[:, t+1, :] = S[t] / (1 + p[t])

Per core (one batch element): bf16 GEMMs z^T [512h, 4096t] (stationary =
W chunk, moving = x^T chunk, 4 rotating PSUM slots of [128,1024]).
Elementwise in [H-partition, T-free]: ACT does Exp/Sigmoid/Reciprocal in
table-batched phases per h-chunk pair (exp -> sig -> recip); DVE does the
g/p STTs and the fp32-accum scan (bf16 out); Pool does the u and o
tensor muls (bf16). Output written bf16, host transposes + upconverts.
"""
import numpy as np

_CACHE = {}

B, T, D, H = 8, 4096, 512, 512
NCORES = 8
N_HC = H // 128       # 4 h-chunks
N_D = D // 128        # 4 contraction chunks
N_TC = 4              # 1024-wide T chunks per h for GEMM+ACT
TCW = 1024
HALF = 2048           # p/u/scan/r2/o granularity
XW_COLS = 3 * H + T   # 5632


def _install_tilefix():
    """This walrus build accepts only ONE sync wait per hardware instruction;
    Tile can emit several. Spill extras onto injected single-wait drains."""
    import concourse.tile as tile
    from concourse import mybir
    from concourse.vector_clock import ScopedClock

    if getattr(tile.TileContext, "_minlstm_patched", False):
        return
    orig_lower = tile.TileContext._lower_ordered_insts

    def _spill_waits(self, ordered):
        nc = self.nc
        for bb_name, insts in ordered.items():
            out = []
            for inst in insts:
                si = inst.sync_info
                if si is not None and len(si.on_wait) > 1 and inst.engine is not None:
                    waits = list(si.on_wait)
                    for w in waits[:-1]:
                        d = mybir.InstDrain(
                            name=nc.get_next_instruction_name(),
                            ins=[], outs=[], bass_is_fusable=False,
                            sync_info=mybir.SyncInfo(on_wait=[w], on_update=[]),
                        )
                        d.engine = inst.engine
                        out.append(d)
                    si.on_wait = [waits[-1]]
                out.append(inst)
            insts[:] = out
        return ordered

    def _patched_lower(self, ordered):
        return orig_lower(self, _spill_waits(self, ordered))

    def _split_drain_and_barrier(self, tick_clock, wait_clock):
        drain_inst = self.nc.sync.drain()
        wait_clock.add_sem_waits(
            drain_inst.ins, ScopedClock({None: tick_clock.global_clock})
        )
        si = drain_inst.ins.sync_info
        if si is not None and len(si.on_wait) > 1:
            waits = list(si.on_wait)
            si.on_wait = [waits[0]]
            for w in waits[1:]:
                extra = self.nc.sync.drain()
                esi = extra.ins.sync_info
                if esi is None:
                    extra.ins.sync_info = mybir.SyncInfo(on_wait=[w], on_update=[])
                else:
                    esi.on_wait = [w]
        self.nc.all_engine_barrier()
        assert self.sems is not None
        popped = self.nc._tile_sem_poison_stack.pop()
        assert popped is self._sem_poison
        self.nc.clear_and_free_semaphores(list(self.sems.allocated().values()))
        self.nc.all_engine_barrier()

    tile.TileContext._lower_ordered_insts = _patched_lower
    tile.TileContext._drain_and_barrier = _split_drain_and_barrier
    tile.TileContext._minlstm_patched = True


def _build():
    import concourse.bass as bass
    import concourse.tile as tile
    from concourse import mybir
    from concourse.tile_rust import add_dep_helper

    _install_tilefix()

    f32 = mybir.dt.float32
    bf16 = mybir.dt.bfloat16
    AF = mybir.ActivationFunctionType
    ALU = mybir.AluOpType

    nc = bass.Bass("TRN2", target_bir_lowering=False, debug=False,
                   num_devices=NCORES)

    xw_d = nc.dram_tensor("xw", [D, XW_COLS], bf16, kind="ExternalInput").ap()
    bias_d = nc.dram_tensor("biases", [128, 16], f32, kind="ExternalInput").ap()
    out_d = nc.dram_tensor("out", [H, T], bf16, kind="ExternalOutput").ap()

    prev_act = [None]

    def act_raw(out, in_, func, bias=0.0, scale=1.0):
        eng = nc.scalar
        inputs = [eng.lower_ap(in_)]
        for arg in (bias, scale, 0.0):
            if isinstance(arg, bass.AP):
                inputs.append(eng.lower_ap(arg))
            else:
                inputs.append(
                    mybir.ImmediateValue(dtype=f32, value=float(arg))
                )
        i = eng.add_instruction(
            mybir.InstActivation(
                name=nc.get_next_instruction_name(),
                func=func, ins=inputs, outs=[eng.lower_ap(out)],
            )
        )
        if prev_act[0] is not None:
            add_dep_helper(i.ins, prev_act[0].ins, sync=False,
                           reason="ACT table-set order")
        prev_act[0] = i
        return i

    with tile.TileContext(nc) as tc:
        with (
            tc.tile_pool(name="xwp", bufs=1) as xwp,
            tc.tile_pool(name="cons", bufs=1) as cons,
            tc.tile_pool(name="ps", bufs=4, space="PSUM") as ps,
            tc.tile_pool(name="grid", bufs=2) as grid,
            tc.tile_pool(name="shp", bufs=4) as shp,
            tc.tile_pool(name="pp", bufs=4) as pp,
            tc.tile_pool(name="up", bufs=4) as up,
            tc.tile_pool(name="Sp", bufs=4) as Sp,
            tc.tile_pool(name="S1kp", bufs=8) as S1kp,
            tc.tile_pool(name="rp", bufs=4) as rp,
            tc.tile_pool(name="op", bufs=4) as op,
        ):
            xw = [
                xwp.tile([128, XW_COLS], bf16, tag=f"xw{d}", name=f"xw{d}")
                for d in range(N_D)
            ]
            # Wf first, then x^T tc0, then Wi/Wh, then remaining x^T chunks
            for d in range(N_D):
                nc.sync.dma_start(
                    xw[d][:, 0:H], xw_d[128 * d:128 * (d + 1), 0:H])
            for d in range(N_D):
                c0 = 3 * H
                nc.sync.dma_start(
                    xw[d][:, c0:c0 + TCW],
                    xw_d[128 * d:128 * (d + 1), c0:c0 + TCW])
            for d in range(N_D):
                nc.sync.dma_start(
                    xw[d][:, H:3 * H], xw_d[128 * d:128 * (d + 1), H:3 * H])
            for tcol in range(1, N_TC):
                c0 = 3 * H + TCW * tcol
                for d in range(N_D):
                    nc.sync.dma_start(
                        xw[d][:, c0:c0 + TCW],
                        xw_d[128 * d:128 * (d + 1), c0:c0 + TCW])
            bt = cons.tile([128, 16], f32, tag="bt")
            nc.sync.dma_start(bt[:], bias_d[:])
            zero1 = cons.tile([128, 8], f32, tag="zero1")
            nc.vector.memset(zero1[:], 0.0)
            zb = zero1[:, 0:1].broadcast_to([128, HALF])
            zb1k = zero1[:, 0:1].broadcast_to([128, TCW])

            def gemm(gate, h, tcol, name):
                z = ps.tile([128, TCW], f32, tag="z", name=name)
                for half in range(2):
                    sl = slice(512 * half, 512 * (half + 1))
                    t0 = 3 * H + TCW * tcol + 512 * half
                    for d in range(N_D):
                        nc.tensor.matmul(
                            z[:, sl], xw[d][:, 512 * gate + 128 * h:
                                            512 * gate + 128 * h + 128],
                            xw[d][:, t0:t0 + 512],
                            start=(d == 0), stop=(d == N_D - 1),
                        )
                return z

            backlog = []

            def drain(n):
                for _ in range(min(n, len(backlog))):
                    backlog.pop(0)()

            for pair in range(2):
                hs = (2 * pair, 2 * pair + 1)
                last = pair == 1

                Ef = {}
                si = {}
                g = {}
                for h in hs:
                    Ef[h] = grid.tile([128, T], bf16, tag="Ef", name=f"Ef{h}")
                    si[h] = grid.tile([128, T], bf16, tag="si", name=f"si{h}")
                    g[h] = grid.tile([128, T], bf16, tag="g", name=f"g{h}")

                # ---- EXP phase: zf GEMMs -> Ef = exp(-zf) ----
                for h in hs:
                    nbf_ap = bt[:, h:h + 1]            # -bf
                    for tcol in range(N_TC):
                        z = gemm(0, h, tcol, f"zf{h}_{tcol}")
                        act_raw(Ef[h][:, TCW * tcol:TCW * (tcol + 1)], z[:],
                                AF.Exp, bias=nbf_ap, scale=-1.0)
                        if tcol % 2 == 1:
                            drain(1)    # prev-pair scan/out piece

                # ---- SIG phase + per-pair postlude ----
                pt = {h: [] for h in hs}
                ut = {h: [] for h in hs}
                St = {h: [] for h in hs}
                rt = {h: [] for h in hs}

                def sig_section(h):
                    bi_ap = bt[:, 4 + h:5 + h]
                    bg_ap = bt[:, 8 + h:9 + h]         # bh + 0.5
                    bh_ap = bt[:, 12 + h:13 + h]
                    for tcol in range(N_TC):
                        zi = gemm(1, h, tcol, f"zi{h}_{tcol}")
                        act_raw(si[h][:, TCW * tcol:TCW * (tcol + 1)], zi[:],
                                AF.Sigmoid, bias=bi_ap)
                        zh = gemm(2, h, tcol, f"zh{h}_{tcol}")
                        sh = shp.tile([128, TCW], bf16, tag="sh",
                                      name=f"sh{h}_{tcol}")
                        act_raw(sh[:], zh[:], AF.Sigmoid, bias=bh_ap)
                        nc.vector.scalar_tensor_tensor(
                            out=g[h][:, TCW * tcol:TCW * (tcol + 1)],
                            in0=zh[:], scalar=bg_ap, in1=sh[:],
                            op0=ALU.add, op1=ALU.max,
                        )
                        if tcol % 2 == 1:
                            drain(1)

                def p_and_u(h):
                    for half in range(2):
                        sl = slice(HALF * half, HALF * (half + 1))
                        p = pp.tile([128, HALF], bf16, tag="p",
                                    name=f"p{h}_{half}")
                        nc.vector.scalar_tensor_tensor(
                            out=p[:], in0=Ef[h][:, sl], scalar=1.0,
                            in1=si[h][:, sl], op0=ALU.add, op1=ALU.mult,
                        )
                        pt[h].append(p)
                    for k in range(N_TC):
                        sl = slice(TCW * k, TCW * (k + 1))
                        u = up.tile([128, TCW], bf16, tag="u1k",
                                    name=f"u1k{h}_{k}")
                        nc.vector.tensor_tensor(
                            out=u[:],
                            in0=pt[h][k // 2][:, TCW * (k % 2):TCW * (k % 2 + 1)],
                            in1=g[h][:, sl], op=ALU.mult)
                        ut[h].append(u)

                def scan_piece(h, k):
                    S = S1kp.tile([128, TCW], bf16, tag="S1k",
                                  name=f"S{h}_{k}")
                    init = 0.5 if k == 0 else St[h][-1][:, TCW - 1:TCW]
                    nc.vector.tensor_tensor_scan(
                        S[:], zb1k, ut[h][k][:], init, ALU.add, ALU.add)
                    St[h].append(S)

                def r2_section(h):
                    for half in range(2):
                        r2 = rp.tile([128, HALF], bf16, tag="r2",
                                     name=f"r2_{h}_{half}")
                        act_raw(r2[:], pt[h][half][:], AF.Reciprocal,
                                bias=1.0)
                        rt[h].append(r2)

                def out_piece(h, k, eng):
                    o = op.tile([128, TCW], bf16, tag="o", name=f"o{h}_{k}")
                    r2sl = rt[h][k // 2][:, TCW * (k % 2):TCW * (k % 2 + 1)]
                    if eng == "v":
                        nc.vector.tensor_tensor(out=o[:], in0=St[h][k][:],
                                                in1=r2sl, op=ALU.mult)
                    else:
                        nc.gpsimd.tensor_tensor(out=o[:], in0=St[h][k][:],
                                                in1=r2sl, op=ALU.mult)
                    nc.sync.dma_start(
                        out_d[128 * h:128 * (h + 1), TCW * k:TCW * (k + 1)],
                        o[:],
                    )

                if not last:
                    for h in hs:
                        sig_section(h)
                    for h in hs:
                        p_and_u(h)
                    for h in hs:
                        r2_section(h)

                    def mk_piece(h, k, ut=ut, St=St, rt=rt):
                        def emit():
                            S = S1kp.tile([128, TCW], bf16, tag="S1k",
                                          name=f"S{h}_{k}")
                            init = (0.5 if k == 0
                                    else St[h][-1][:, TCW - 1:TCW])
                            nc.vector.tensor_tensor_scan(
                                S[:], zb1k, ut[h][k][:], init,
                                ALU.add, ALU.add)
                            St[h].append(S)
                            o = op.tile([128, TCW], bf16, tag="o",
                                        name=f"o{h}_{k}")
                            r2sl = rt[h][k // 2][:, TCW * (k % 2):
                                                 TCW * (k % 2 + 1)]
                            nc.gpsimd.tensor_tensor(out=o[:], in0=St[h][k][:],
                                                    in1=r2sl, op=ALU.mult)
                            nc.sync.dma_start(
                                out_d[128 * h:128 * (h + 1),
                                      TCW * k:TCW * (k + 1)],
                                o[:],
                            )
                        return emit

                    for h in hs:
                        for k in range(N_TC):
                            backlog.append(mk_piece(h, k))
                else:
                    h2, h3 = hs
                    # h2: postlude inline; its scans run under h3's GEMMs
                    sig_section(h2)
                    p_and_u(h2)
                    for k in range(N_TC):
                        scan_piece(h2, k)
                    # h3: all zi first (si ready early), then zh with the
                    # g/u/scan chain chasing each 1024-chunk
                    bi_ap = bt[:, 4 + h3:5 + h3]
                    bg_ap = bt[:, 8 + h3:9 + h3]
                    bh_ap = bt[:, 12 + h3:13 + h3]
                    for tcol in range(N_TC):
                        zi = gemm(1, h3, tcol, f"zi{h3}_{tcol}")
                        act_raw(si[h3][:, TCW * tcol:TCW * (tcol + 1)],
                                zi[:], AF.Sigmoid, bias=bi_ap)
                        if tcol % 2 == 1:
                            drain(1)
                    for half in range(2):
                        sl = slice(HALF * half, HALF * (half + 1))
                        p = pp.tile([128, HALF], bf16, tag="p",
                                    name=f"p{h3}_{half}")
                        nc.vector.scalar_tensor_tensor(
                            out=p[:], in0=Ef[h3][:, sl], scalar=1.0,
                            in1=si[h3][:, sl], op0=ALU.add, op1=ALU.mult,
                        )
                        pt[h3].append(p)
                    for tcol in range(N_TC):
                        zh = gemm(2, h3, tcol, f"zh{h3}_{tcol}")
                        sh = shp.tile([128, TCW], bf16, tag="sh",
                                      name=f"sh{h3}_{tcol}")
                        act_raw(sh[:], zh[:], AF.Sigmoid, bias=bh_ap)
                        nc.vector.scalar_tensor_tensor(
                            out=g[h3][:, TCW * tcol:TCW * (tcol + 1)],
                            in0=zh[:], scalar=bg_ap, in1=sh[:],
                            op0=ALU.add, op1=ALU.max,
                        )
                        u = up.tile([128, TCW], bf16, tag="u1k",
                                    name=f"u1k{h3}_{tcol}")
                        nc.vector.tensor_tensor(
                            out=u[:],
                            in0=pt[h3][tcol // 2][:, TCW * (tcol % 2):
                                                  TCW * (tcol % 2 + 1)],
                            in1=g[h3][:, TCW * tcol:TCW * (tcol + 1)],
                            op=ALU.mult)
                        ut[h3].append(u)
                        scan_piece(h3, tcol)
                        if tcol % 2 == 1:
                            drain(1)
                    # recips: h3 first (gates the tail), then h2
                    r2_section(h3)
                    r2_section(h2)
                    for k in range(N_TC):
                        out_piece(h3, k, "v")
                    for k in range(N_TC):
                        out_piece(h2, k, "v")
            drain(len(backlog))
    return nc


def _get_nc():
    if "nc" not in _CACHE:
        _CACHE["nc"] = _build()
    return _CACHE["nc"]


def _make_in_maps(x, Wf, bf, Wi, bi, Wh, bh):
    import ml_dtypes
    bft = ml_dtypes.bfloat16

    x = np.asarray(x, dtype=np.float32)
    W_all = np.concatenate(
        [np.asarray(Wf), np.asarray(Wi), np.asarray(Wh)], axis=1
    ).astype(bft)

    bf32 = np.asarray(bf, dtype=np.float32)
    bi32 = np.asarray(bi, dtype=np.float32)
    bh32 = np.asarray(bh, dtype=np.float32)
    biases = np.zeros((128, 16), dtype=np.float32)
    biases[:, 0:4] = (-bf32).reshape(N_HC, 128).T
    biases[:, 4:8] = bi32.reshape(N_HC, 128).T
    biases[:, 8:12] = (bh32 + np.float32(0.5)).reshape(N_HC, 128).T
    biases[:, 12:16] = bh32.reshape(N_HC, 128).T

    in_maps = []
    for c in range(NCORES):
        xT = np.ascontiguousarray(x[c].T).astype(bft)
        xw = np.concatenate([W_all, xT], axis=1)
        in_maps.append({"xw": xw, "biases": biases})
    return in_maps


def kernel(x, Wf, bf, Wi, bi, Wh, bh):
    from concourse.bass_utils import run_bass_kernel_spmd

    in_maps = _make_in_maps(x, Wf, bf, Wi, bi, Wh, bh)
    nc = _get_nc()
    res = run_bass_kernel_spmd(nc, in_maps, list(range(NCORES)))

    out = np.empty((B, T + 1, H), dtype=np.float32)
    out[:, 0, :] = np.float32(0.5)
    for c in range(NCORES):
        out[c, 1:, :] = np.asarray(res.results[c]["out"]).astype(np.float32).T
    return out
